# revision 13
# baseline (speedup 1.0000x reference)
"""Distributed causal multi-head attention for TRN2 (8 NeuronCores).

Problem: x[2,2048,1024], w_qkv[1024,16,192], w_out[16,64,1024] (biases zero).
Sharding: 2 batch groups x 4-way tensor-parallel over heads (4 heads/core).
Per core: QKV projection fused chunk-wise into causal flash-style attention
(attention for q-chunk r only needs x-chunks 0..r), 2-head PE-array packing
(row-split scores, col-split AV), output projection, then a chunked bf16
ReduceScatter over each 4-core group; each core keeps a disjoint slice of
the final output so host-side unsharding is a pure gather (+f32 cast).

bf16 matmuls throughout (fp32 matmul is 2-pass on TRN2 = 1.9x slower);
softmax denominator via a ones-matmul that also broadcasts across
partitions; exp for both heads of a pair in one ACT instruction over a
two-bank PSUM tile; causal masks are precomputed tiles applied on DVE.
"""

import numpy as np

BS, S_FULL, D, H = 2, 2048, 1024, 16
DH = 64
P = 128
HL = 4              # heads per core
QCW = 512           # q-chunk width
NCORE = 8
GROUPS = [[0, 1, 2, 3], [4, 5, 6, 7]]
# ReduceScatter chunks in q-tile (128-row) units; last 512 rows split in two
# so the final collective starts earlier and the exposed tail is shorter.
RS_CH = [(0, 4), (4, 8), (8, 12), (12, 16)]

_CACHE = {}


def build_graph(S=S_FULL):
    """Build the SPMD single-core graph (same on all 8 cores)."""
    import concourse.bacc as bacc
    import concourse.mybir as mybir
    import concourse.tile as tile

    F32 = mybir.dt.float32
    BF16 = mybir.dt.bfloat16
    Act = mybir.ActivationFunctionType
    Alu = mybir.AluOpType

    NDT = D // P                 # 8 d-tiles (contraction of qkv proj)
    NMC = S // QCW               # m-chunks of x / q-chunks
    NQC = NMC
    COT = (HL * DH) // P         # 2 c-tiles (contraction of out proj)
    NFT = 2 * HL * DH // P       # 4 qk feature tiles
    GW = len(GROUPS[0])

    nc = bacc.Bacc("TRN2", target_bir_lowering=False, debug=False,
                   num_devices=NCORE)

    xt_ext = nc.dram_tensor("xt", [P, NDT, S], F32, kind="ExternalInput")
    wqk_ext = nc.dram_tensor("wqk", [P, NDT, 2 * HL * DH], F32, kind="ExternalInput")
    wv_ext = nc.dram_tensor("wv", [P, NDT, HL * DH], F32, kind="ExternalInput")
    wout_ext = nc.dram_tensor("wout", [P, COT, D], F32, kind="ExternalInput")
    bqk_ext = nc.dram_tensor("bqk", [NFT, P], F32, kind="ExternalInput")
    bv_ext = nc.dram_tensor("bv", [1, HL * DH], F32, kind="ExternalInput")
    bout4_ext = nc.dram_tensor("bout4", [1, D], F32, kind="ExternalInput")
    out_ext = nc.dram_tensor("out", [S // GW, D], BF16, kind="ExternalOutput")

    with tile.TileContext(nc) as tc:
        with (
            tc.tile_pool(name="persist", bufs=1) as pp,
            tc.tile_pool(name="xchunk", bufs=1) as xp,
            tc.tile_pool(name="pt", bufs=6) as ptp,
            tc.tile_pool(name="recip", bufs=2) as rcp,
            tc.tile_pool(name="outsb", bufs=4) as osp,
            tc.tile_pool(name="ps", bufs=1, space="PSUM") as ps,
            tc.tile_pool(name="dram", bufs=1, space="DRAM") as dp,
        ):
            # ---- persistent SBUF tensors ----
            wqk_sb = pp.tile([P, NDT * 512], BF16, name="wqk_sb")
            wv_sb = pp.tile([P, NDT * 256], BF16, name="wv_sb")
            wout_sb = pp.tile([P, COT * D], BF16, name="wout_sb")
            bqk_sb = pp.tile([P, NFT], F32, name="bqk_sb")
            bv_row = pp.tile([1, 256], F32, name="bv_row")
            bvb_sb = pp.tile([P, 256], F32, name="bvb_sb")
            bob_row = pp.tile([1, D], F32, name="bob_row")
            bob_sb = pp.tile([P, D], F32, name="bob_sb")
            ones_sb = pp.tile([P, DH], BF16, name="ones_sb")
            warm_sb = pp.tile([4, DH], BF16, name="warm_sb")
            qkT = [pp.tile([P, S], BF16, name=f"qkT{ft}") for ft in range(NFT)]
            v_sb = pp.tile([P, (S // P) * 256], BF16, name="v_sb")
            attnT = [pp.tile([P, S], BF16, name=f"attnT{ct}") for ct in range(COT)]
            # doubled causal mask tiles, one per diagonal offset j:
            # masks[j][kk, sub, qq] = 1 if qq >= kk + j*128 else 0
            masks = [pp.tile([P, 2 * QCW], BF16, name=f"mask{j}")
                     for j in range(QCW // P)]

            # ---- DRAM bounce buffers for the collectives (bf16) ----
            rs_in = [dp.tile([(e - s) * P, D], BF16, name=f"rs_in{ci}")
                     for ci, (s, e) in enumerate(RS_CH)]
            rs_out = [dp.tile([(e - s) * P // GW, D], BF16, name=f"rs_out{ci}")
                      for ci, (s, e) in enumerate(RS_CH)]
            warm_in = dp.tile([4, DH], BF16, name="warm_in")
            warm_out = dp.tile([1, DH], BF16, name="warm_out")

            # ---- loads (f32 -> bf16 cast during SWDGE DMA), criticals first
            for ft in range(NFT):
                nc.sync.dma_start(out=bqk_sb[:, ft:ft + 1],
                                  in_=bqk_ext[ft:ft + 1, :].rearrange("o p -> p o"))
            nc.sync.dma_start(out=bv_row[:], in_=bv_ext[:])
            nc.sync.dma_start(out=bob_row[:], in_=bout4_ext[:])
            nc.vector.memset(ones_sb[:], 1.0)
            nc.vector.memset(warm_sb[:], 1.0)
            nc.sync.dma_start(out=warm_in[:], in_=warm_sb[:])
            for j in range(QCW // P):
                nc.vector.memset(masks[j][:], 1.0)

            # everything round 0 needs comes first: wqk, xch0, wv, biases,
            # masks. wqk/xch0 ride fast HWDGE f32 loads + DVE bf16 casts.
            xchs = [xp.tile([P, NDT * QCW], BF16, name=f"xch{mc}", tag=f"x{mc}")
                    for mc in range(NMC)]
            wqk_f32 = xp.tile([P, NDT * 512], F32, name="wqk_f32", tag="st0")
            xch0_f32 = xp.tile([P, NDT * QCW], F32, name="xch0_f32", tag="st1")
            nc.sync.dma_start(
                out=wqk_f32[:].rearrange("p (d f) -> p d f", d=NDT),
                in_=wqk_ext[:])
            nc.sync.dma_start(
                out=xch0_f32[:].rearrange("p (d m) -> p d m", d=NDT),
                in_=xt_ext[:, :, 0:QCW])
            hd = NDT // 2
            for half in range(2):
                sl = slice(half * hd * 512, (half + 1) * hd * 512)
                nc.vector.tensor_copy(wqk_sb[:, sl], wqk_f32[:, sl])
                nc.vector.tensor_copy(xchs[0][:, sl], xch0_f32[:, sl])
            nc.gpsimd.dma_start(
                out=wv_sb[:].rearrange("p (d f) -> p d f", d=NDT),
                in_=wv_ext[:])
            nc.gpsimd.partition_broadcast(bvb_sb[:], bv_row[:])
            nc.gpsimd.partition_broadcast(bob_sb[:], bob_row[:])
            for j in range(QCW // P):
                nc.gpsimd.affine_select(
                    masks[j][:].rearrange("p (s w) -> p s w", s=2),
                    masks[j][:].rearrange("p (s w) -> p s w", s=2),
                    pattern=[[0, 2], [1, QCW]], compare_op=Alu.is_ge,
                    fill=0.0, base=-j * P, channel_multiplier=-1)
            # warm up the collective engine while loads run
            nc.gpsimd.collective_compute(
                "ReduceScatter", Alu.add, replica_groups=GROUPS,
                ins=[warm_in[:]], outs=[warm_out[:]])
            for mc in range(1, NMC):
                nc.gpsimd.dma_start(
                    out=xchs[mc][:].rearrange("p (d m) -> p d m", d=NDT),
                    in_=xt_ext[:, :, mc * QCW:(mc + 1) * QCW])
                if mc == 1:
                    nc.gpsimd.dma_start(
                        out=wout_sb[:].rearrange("p (c f) -> p c f", c=COT),
                        in_=wout_ext[:])

            # ---- projection work units (one x-chunk = 4 qk + 4 v units) ----
            def do_qk(mc, ft):
                xch = xchs[mc]
                pqk = ps.tile([P, 512], F32, name="pqk", tag="pv", bufs=2)
                for d in range(NDT):
                    nc.tensor.matmul(
                        pqk[:],
                        wqk_sb[:, d * 512 + ft * P:d * 512 + (ft + 1) * P],
                        xch[:, d * QCW:(d + 1) * QCW],
                        start=(d == 0), stop=(d == NDT - 1))
                nc.vector.tensor_scalar_add(
                    qkT[ft][:, mc * QCW:(mc + 1) * QCW], pqk[:],
                    bqk_sb[:, ft:ft + 1])

            def do_v(mc, mt):
                xch = xchs[mc]
                gmt = mc * (QCW // P) + mt
                pv = ps.tile([P, 256], F32, name="pv", tag="pv", bufs=2)
                for d in range(NDT):
                    nc.tensor.matmul(
                        pv[:],
                        xch[:, d * QCW + mt * P:d * QCW + (mt + 1) * P],
                        wv_sb[:, d * 256:(d + 1) * 256],
                        start=(d == 0), stop=(d == NDT - 1))
                nc.vector.tensor_add(v_sb[:, gmt * 256:(gmt + 1) * 256],
                                     pv[:], bvb_sb[:])

            def proj_units(mc):
                return ([(do_qk, mc, ft) for ft in range(NFT)] +
                        [(do_v, mc, mt) for mt in range(QCW // P)])

            # chunk 0 projection up front
            for fn, a1, a2 in proj_units(0):
                fn(a1, a2)

            # ---- fused rounds: attention(qc=r) + interleaved proj(r+1) ----
            out_row_off = 0
            for r in range(NQC):
                units = proj_units(r + 1) if r + 1 < NMC else []
                ui = 0
                nkt = (r + 1) * (QCW // P)
                steps_total = 2 * nkt
                step = 0
                q0 = r * QCW
                for pr in range(HL // 2):        # head pairs (2pr, 2pr+1)
                    qt_t = qkT[pr]
                    kt_t = qkT[2 + pr]
                    av = ps.tile([P, QCW], F32, name="av", tag="av", bufs=1)
                    den = ps.tile([P, QCW], F32, name="den", tag="den", bufs=1)

                    def av_den(kt, pt2, a, first, last):
                        for sub in (0, 1):
                            h = 2 * pr + sub
                            nc.tensor.matmul(
                                av[sub * DH:(sub + 1) * DH, a:QCW],
                                v_sb[:, kt * 256 + h * DH:kt * 256 + (h + 1) * DH],
                                pt2[:, sub * QCW + a:(sub + 1) * QCW],
                                start=first, stop=last, skip_group_check=True)
                        for sub in (0, 1):
                            nc.tensor.matmul(
                                den[sub * DH:(sub + 1) * DH, a:QCW],
                                ones_sb[:],
                                pt2[:, sub * QCW + a:(sub + 1) * QCW],
                                start=first, stop=last, skip_group_check=True)

                    pending = []
                    for kt in range(nkt):
                        off = max(0, (kt - 4 * r) * P)
                        a = off
                        s2 = ps.tile([P, 2 * QCW], F32, name="s2", tag="s",
                                     bufs=2)
                        for sub in (0, 1):
                            bp = sub * DH
                            nc.tensor.matmul(
                                s2[:, sub * QCW + a:(sub + 1) * QCW],
                                kt_t[bp:bp + DH, kt * P:(kt + 1) * P],
                                qt_t[bp:bp + DH, q0 + a:q0 + QCW],
                                start=True, stop=True)
                        pt2 = ptp.tile([P, 2 * QCW], BF16, name="pt2", tag="pt")
                        s2v = s2[:].rearrange("p (s w) -> p s w", s=2)
                        pt2v = pt2[:].rearrange("p (s w) -> p s w", s=2)
                        nc.scalar.activation(pt2v[:, :, a:QCW], s2v[:, :, a:QCW],
                                             Act.Exp)
                        if kt >= 4 * r:
                            j = kt - 4 * r
                            wlen = min(off + P, QCW) - a
                            mv = masks[j][:].rearrange("p (s w) -> p s w", s=2)
                            nc.vector.tensor_mul(pt2v[:, :, a:a + wlen],
                                                 pt2v[:, :, a:a + wlen],
                                                 mv[:, :, a:a + wlen])
                        # pace projection units of the next x-chunk into gaps
                        if units and ui < len(units) and \
                                step * len(units) >= ui * steps_total:
                            fn, a1, a2 = units[ui]
                            fn(a1, a2)
                            ui += 1
                        pending.append((kt, pt2, a))
                        if len(pending) > 2:
                            pv_ = pending.pop(0)
                            av_den(*pv_, first=(pv_[0] == 0), last=False)
                        step += 1
                    while pending:
                        pv_ = pending.pop(0)
                        av_den(*pv_, first=(pv_[0] == 0),
                               last=(not pending))

                    recip = rcp.tile([P, QCW], F32, name="recip", tag="rc")
                    nc.vector.reciprocal_approx_fast(recip[:], den[:])
                    nc.vector.tensor_mul(attnT[pr][:, q0:q0 + QCW], av[:],
                                         recip[:])
                while ui < len(units):
                    fn, a1, a2 = units[ui]
                    fn(a1, a2)
                    ui += 1

                # out projection for this q-chunk + any completed RS chunks
                for qi in range(QCW // P):
                    qt_g = r * (QCW // P) + qi
                    outsb = osp.tile([P, D], BF16, name="outsb", tag="ot")
                    for oc in range(D // 512):
                        po = ps.tile([P, 512], F32, name="po", tag="pv",
                                     bufs=2)
                        for ct in range(COT):
                            nc.tensor.matmul(
                                po[:],
                                attnT[ct][:, qt_g * P:(qt_g + 1) * P],
                                wout_sb[:, ct * D + oc * 512:ct * D + (oc + 1) * 512],
                                start=(ct == 0), stop=(ct == COT - 1))
                        nc.vector.tensor_add(outsb[:, oc * 512:(oc + 1) * 512],
                                             po[:],
                                             bob_sb[:, oc * 512:(oc + 1) * 512])
                    for ci, (cs, ce) in enumerate(RS_CH):
                        if cs <= qt_g < ce:
                            # scalar-engine HWDGE queue: keeps the sync queue
                            # free for the RS-gated final output copies
                            nc.scalar.dma_start(
                                out=rs_in[ci][(qt_g - cs) * P:(qt_g - cs + 1) * P, :],
                                in_=outsb[:])
                    for ci, (cs, ce) in enumerate(RS_CH):
                        if ce == qt_g + 1:
                            nc.gpsimd.collective_compute(
                                "ReduceScatter", Alu.add, replica_groups=GROUPS,
                                ins=[rs_in[ci][:]], outs=[rs_out[ci][:]])

            # final output copies (bf16), off the critical path on sync
            row = 0
            for ci, (cs, ce) in enumerate(RS_CH):
                rows = (ce - cs) * P // GW
                nc.sync.dma_start(out=out_ext[row:row + rows, :],
                                  in_=rs_out[ci][:])
                row += rows

    nc.compile()
    return nc


def shard_inputs(x, w_qkv, b_qkv, w_out, b_out, S=S_FULL):
    """Host-side sharding: per-core input dicts (pure layout work)."""
    scale = np.float32(DH ** -0.5)
    x = np.asarray(x, dtype=np.float32)
    w_qkv = np.asarray(w_qkv, dtype=np.float32)
    b_qkv = np.asarray(b_qkv, dtype=np.float32)
    w_out = np.asarray(w_out, dtype=np.float32)
    b_out = np.asarray(b_out, dtype=np.float32)
    NDT = D // P
    in_maps = []
    for c in range(NCORE):
        g, r = c // 4, c % 4
        hs = slice(HL * r, HL * (r + 1))
        xt = np.ascontiguousarray(
            x[g, :S].T.reshape(NDT, P, S).transpose(1, 0, 2))
        wq = (w_qkv[:, hs, 0:DH] * scale).reshape(D, HL * DH)
        wk = w_qkv[:, hs, DH:2 * DH].reshape(D, HL * DH)
        wqk = np.ascontiguousarray(
            np.concatenate([wq, wk], axis=1).reshape(NDT, P, 2 * HL * DH)
            .transpose(1, 0, 2))
        wv = np.ascontiguousarray(
            w_qkv[:, hs, 2 * DH:3 * DH].reshape(D, HL * DH)
            .reshape(NDT, P, HL * DH).transpose(1, 0, 2))
        wo = np.ascontiguousarray(
            w_out[hs].reshape(HL * DH, D).reshape(2, P, D).transpose(1, 0, 2))
        bq = (b_qkv[hs, 0:DH] * scale).reshape(HL * DH)
        bk = b_qkv[hs, DH:2 * DH].reshape(HL * DH)
        bqk = np.concatenate([bq, bk]).reshape(2 * HL * DH // P, P)
        bv = b_qkv[hs, 2 * DH:3 * DH].reshape(1, HL * DH)
        bout4 = (b_out / len(GROUPS[0])).reshape(1, D)
        in_maps.append({
            "xt": np.ascontiguousarray(xt),
            "wqk": wqk, "wv": wv, "wout": wo,
            "bqk": np.ascontiguousarray(bqk),
            "bv": np.ascontiguousarray(bv),
            "bout4": np.ascontiguousarray(bout4),
        })
    return in_maps


def unshard_output(results, S=S_FULL):
    """Pure gather of per-core RS slices into the full output (+f32 cast)."""
    import ml_dtypes
    out = np.empty((BS, S, D), dtype=np.float32)
    GW = len(GROUPS[0])
    for c in range(NCORE):
        g, r = c // 4, c % 4
        o = results[c]["out"]          # [S//GW, D] bf16 (viewed as uint16?)
        if o.dtype != np.float32:
            o = o.view(ml_dtypes.bfloat16).astype(np.float32) \
                if o.dtype == np.uint16 else o.astype(np.float32)
        else:
            o = o
        row = 0
        for (cs, ce) in RS_CH:
            rows = (ce - cs) * P // GW
            out[g, cs * P + r * rows:cs * P + (r + 1) * rows] = o[row:row + rows]
            row += rows
    return out


def kernel(x, w_qkv, b_qkv, w_out, b_out, trace=False):
    from concourse.bass_utils import run_bass_kernel_spmd
    if "nc" not in _CACHE:
        _CACHE["nc"] = build_graph()
    nc = _CACHE["nc"]
    in_maps = shard_inputs(x, w_qkv, b_qkv, w_out, b_out)
    res = run_bass_kernel_spmd(nc, in_maps, core_ids=list(range(NCORE)),
                               trace=trace)
    _CACHE["last_results"] = res
    return unshard_output(res.results)
